# revision 16
# baseline (speedup 1.0000x reference)
"""Trainium2 Bass kernel: 16-head MHA (B=2, T=2048, D=1024), head-TP over 8 cores.

Per core c: heads 2c, 2c+1 (128 channels). Device computes x@Wqkv(+b) for its
head slice, scoresT=K@Q^T (scale folded into Wq), exp via ACT, P@V with an
appended ones-column producing the softmax denominator for free, normalize,
then partial proj = attn_c @ Wproj[c-slice]. Host sums the 8 partials + b_proj.

v2: proj deferred one pipeline unit (off the normalize critical path), h1
normalize written directly to attnT partitions 64:128 (no SBUF-SBUF DMA),
a2 bias-adds moved off the exp-saturated ACT engine to gpsimd, proj PSUM
drains split vector/gpsimd, startup DMAs reordered onto the two HWDGE rings,
out-tile DMAs alternate the HWDGE rings.
"""

import numpy as np
import ml_dtypes
from contextlib import ExitStack

B, T, C = 2, 2048, 1024
H, DH = 16, 64
NCORES = 8
CH = 128               # channels per core = 2 heads
NTOK = B * T           # 4096
NKC = T // 128         # 16 key chunks per batch
NQC = T // 512         # 4 query chunks per batch
SCALE = DH ** -0.5

_CACHE = {}


def _build(debug=False):
    import concourse.bass as bass  # noqa: F401
    import concourse.bacc as bacc
    import concourse.mybir as mybir
    import concourse.tile as tile

    f32 = mybir.dt.float32
    bf16 = mybir.dt.bfloat16
    EXP = mybir.ActivationFunctionType.Exp
    IDENT = mybir.ActivationFunctionType.Identity

    # Bacc (not Bass): its compile() runs move_matmul_waits_to_ldweights +
    # generate_event_semaphores, without which walrus rejects matmuls
    # carrying 2 sync waits ("Too many sync wait commands").
    nc = bacc.Bacc("TRN2", target_bir_lowering=False, debug=False)
    xT_d = nc.declare_dram_parameter("xT", [C, NTOK], bf16, isOutput=False)
    wq_d = nc.declare_dram_parameter("wq", [128, C], bf16, isOutput=False)
    wk_d = nc.declare_dram_parameter("wk", [128, C], bf16, isOutput=False)
    wv_d = nc.declare_dram_parameter("wv", [128, C], bf16, isOutput=False)
    wp_d = nc.declare_dram_parameter("wp", [CH, C], bf16, isOutput=False)
    bqc_d = nc.declare_dram_parameter("bqc", [CH, 1], f32, isOutput=False)
    bkc_d = nc.declare_dram_parameter("bkc", [CH, 1], f32, isOutput=False)
    bv_d = nc.declare_dram_parameter("bv", [1, CH], bf16, isOutput=False)
    out_d = nc.declare_dram_parameter("out", [NTOK, C], bf16, isOutput=True)

    with tile.TileContext(nc) as tc, ExitStack() as ctx:
        ep = ctx.enter_context

        # ---------------- persistent SBUF ----------------
        xT_pool = ep(tc.tile_pool(name="xT", bufs=8))
        xT_sb = [xT_pool.tile([128, NTOK], bf16, name=f"xT{k}", tag="xT") for k in range(8)]
        w_pool = ep(tc.tile_pool(name="w", bufs=4))
        wq_sb = w_pool.tile([128, C], bf16, tag="wq")
        wk_sb = w_pool.tile([128, C], bf16, tag="wk")
        wv_sb = w_pool.tile([128, C], bf16, tag="wv")
        wp_sb = w_pool.tile([CH, C], bf16, tag="wp")
        b_pool = ep(tc.tile_pool(name="bias", bufs=1))
        bqc_sb = b_pool.tile([CH, 1], f32, tag="bqc")
        bkc_sb = b_pool.tile([CH, 1], f32, tag="bkc")
        bv_sb = b_pool.tile([1, CH], bf16, tag="bv")
        bv_bc = b_pool.tile([128, CH], bf16, tag="bv_bc")
        zeros_col = b_pool.tile([128, 1], f32, tag="zeros_col")
        const_pool = ep(tc.tile_pool(name="const", bufs=2))
        ones_bf = const_pool.tile([1, 512], bf16, tag="ones_bf")
        qk_pool = ep(tc.tile_pool(name="qk", bufs=2))
        qT_sb = qk_pool.tile([CH, NTOK], bf16, tag="qT")
        kT_sb = qk_pool.tile([CH, NTOK], bf16, tag="kT")
        v_pool = ep(tc.tile_pool(name="v", bufs=2))
        # per head: B*NKC chunks of [128 keys, 64 ones cols | 64 feats]; the
        # ones cols make the PV matmul replicate the softmax denominator onto
        # output partitions 0:64 for free (recip reads physical partition 0).
        v_sb = [v_pool.tile([128, B * NKC * 128], bf16, name=f"v{h}", tag="v") for h in range(2)]
        attn_pool = ep(tc.tile_pool(name="attn", bufs=2))
        attnT = [attn_pool.tile([CH, T], bf16, name=f"attnT{b}", tag="attnT") for b in range(B)]
        exp_pool = ep(tc.tile_pool(name="exp", bufs=6))
        bc_pool = ep(tc.tile_pool(name="bcsb", bufs=2))
        out_pool = ep(tc.tile_pool(name="outsb", bufs=4))

        # ---------------- memsets first (no DMA deps) ----------------
        nc.vector.memset(ones_bf[:], 1.0)
        nc.vector.memset(zeros_col[:], 0.0)
        # whole v tile to 1.0; value cols 0:64 of each chunk overwritten later
        nc.vector.memset(v_sb[0][:], 1.0)
        nc.gpsimd.memset(v_sb[1][:], 1.0)

        # ---------------- load inputs ----------------
        # x loads as [128, 2048] per-batch blocks: 4KB contiguous lines per
        # partition (near-peak DMA efficiency) and only 16 transfers.  Batch 0
        # first (phase A consumes it), batch 1 right after (a2 jobs need it
        # from ~45us).  All on the two HWDGE rings; weights interleaved.
        def xblock(q, k, b):
            q.dma_start(
                xT_sb[k][:, b * T:(b + 1) * T],
                xT_d[k * 128:(k + 1) * 128, b * T:(b + 1) * T])

        nc.scalar.dma_start(bv_sb[:], bv_d[:])
        nc.sync.dma_start(wq_sb[:, 0:512], wq_d[:, 0:512])
        nc.scalar.dma_start(wq_sb[:, 512:1024], wq_d[:, 512:1024])
        nc.scalar.dma_start(bqc_sb[:], bqc_d[:])
        nc.scalar.dma_start(bkc_sb[:], bkc_d[:])
        for k in range(8):
            xblock(nc.sync if k % 2 == 0 else nc.scalar, k, 0)
        nc.sync.dma_start(wk_sb[:], wk_d[:])
        nc.scalar.dma_start(wv_sb[:], wv_d[:])
        for k in range(8):
            xblock(nc.sync if k % 2 == 0 else nc.scalar, k, 1)
        nc.sync.dma_start(wp_sb[:], wp_d[:])

        # ---------------- phase A: qkv projections ----------------
        with tc.tile_pool(name="qk_ps", bufs=2, space="PSUM") as qk_psp, \
             tc.tile_pool(name="v_psp", bufs=4, space="PSUM") as v_psp:
            # bv broadcast [128, CH] built once (v bias folded into DVE copy)
            bvps = v_psp.tile([128, CH], f32, name="bv_ps", tag="v_ps")
            nc.tensor.matmul(bvps[:], lhsT=ones_bf[:, :128], rhs=bv_sb[:],
                             start=True, stop=True)
            nc.vector.tensor_copy(bv_bc[:], bvps[:])
            # per 1024-token group: q chunk, k chunk, then 8 v chunks.
            # only batch 0 (t=0,1) here; batch 1's qkv is interleaved into the
            # ACT-bound b=0 attention window below.
            for t in range(2):
                for w_sb, bias_col, dst in ((wq_sb, bqc_sb, qT_sb),
                                            (wk_sb, bkc_sb, kT_sb)):
                    ps = qk_psp.tile([CH, 1024], f32, name="qk_ps", tag="qk_ps")
                    for half in range(2):
                        for k in range(8):
                            nc.tensor.matmul(
                                ps[:, half * 512:(half + 1) * 512],
                                lhsT=w_sb[:, k * 128:(k + 1) * 128],
                                rhs=xT_sb[k][:, t * 1024 + half * 512:
                                              t * 1024 + (half + 1) * 512],
                                start=(k == 0), stop=(k == 7))
                    # bias add fused into PSUM->SBUF copy on the (here idle)
                    # ACT engine
                    nc.scalar.activation(
                        dst[:, t * 1024:(t + 1) * 1024], ps[:], IDENT,
                        bias=bias_col[:])
                for tt in range(t * 8, (t + 1) * 8):
                    ps = v_psp.tile([128, CH], f32, name="v_ps", tag="v_ps")
                    for k in range(8):
                        nc.tensor.matmul(
                            ps[:], lhsT=xT_sb[k][:, tt * 128:(tt + 1) * 128],
                            rhs=wv_sb[:, k * 128:(k + 1) * 128],
                            start=(k == 0), stop=(k == 7))
                    for h in range(2):
                        nc.vector.tensor_add(
                            v_sb[h][:, tt * 128 + 64:(tt + 1) * 128],
                            ps[:, h * 64:(h + 1) * 64],
                            bv_bc[:, h * 64:(h + 1) * 64])

        # ---------------- phase B: attention (+ interleaved proj of prev unit)
        with tc.tile_pool(name="scores_ps", bufs=2, space="PSUM") as scores_ps, \
             tc.tile_pool(name="pv_ps", bufs=2, space="PSUM") as pv_ps, \
             tc.tile_pool(name="proj_ps", bufs=2, space="PSUM") as proj_ps:

            def emit_proj_tci(b, tci):
                """proj partial for one 128-token chunk: out += attn @ Wp_c"""
                osb = out_pool.tile([128, 1024], bf16, name="out_sb", tag="out_sb")
                for ncol in range(2):
                    pps = proj_ps.tile([128, 512], f32, name="proj_ps", tag="pj")
                    nc.tensor.matmul(
                        pps[:],
                        lhsT=attnT[b][:, tci * 128:(tci + 1) * 128],
                        rhs=wp_sb[:, ncol * 512:(ncol + 1) * 512],
                        start=True, stop=True)
                    nc.vector.tensor_copy(
                        osb[:, ncol * 512:(ncol + 1) * 512], pps[:])
                oq = nc.gpsimd if tci % 2 == 0 else nc.sync
                oq.dma_start(
                    out_d[b * T + tci * 128: b * T + (tci + 1) * 128, :], osb[:])

            def emit_a2_qk(w_sb, bias_col, dst, t, half):
                """one [CH,512] half of a batch-1 q/k projection group"""
                ps = proj_ps.tile([128, 512], f32, name="a2_ps", tag="pj")
                sl = slice(t * 1024 + half * 512, t * 1024 + (half + 1) * 512)
                for k in range(8):
                    nc.tensor.matmul(
                        ps[:], lhsT=w_sb[:, k * 128:(k + 1) * 128],
                        rhs=xT_sb[k][:, sl], start=(k == 0), stop=(k == 7))
                # bias add on vector: ACT is exp-saturated in this window
                # (gpsimd cannot read PSUM)
                nc.vector.tensor_scalar_add(dst[:, sl], ps[:], bias_col[:])

            def emit_a2_v(tt):
                """one [128,CH] batch-1 v chunk"""
                ps = proj_ps.tile([128, 512], f32, name="a2v_ps", tag="pj")
                for k in range(8):
                    nc.tensor.matmul(
                        ps[:, 0:CH], lhsT=xT_sb[k][:, tt * 128:(tt + 1) * 128],
                        rhs=wv_sb[:, k * 128:(k + 1) * 128],
                        start=(k == 0), stop=(k == 7))
                for h in range(2):
                    nc.vector.tensor_add(
                        v_sb[h][:, tt * 128 + 64:(tt + 1) * 128],
                        ps[:, h * 64:(h + 1) * 64],
                        bv_bc[:, h * 64:(h + 1) * 64])

            # Units u0..u7 = (b, qc).  Globally software-pipelined stream:
            # PV of unit u lags its scores by 4 kc and its last 4 PVs drain
            # in the first two iterations of unit u+1; exp tiles buffer 6
            # deep so the ACT engine never starves across unit boundaries.
            # a2 (batch-1 qkv) and deferred proj jobs are distributed so
            # every unit's tensor work exceeds its 16us of exp on ACT.
            units = [(b, qc) for b in range(B) for qc in range(NQC)]
            QT2 = (wq_sb, bqc_sb, qT_sb, 2)
            KT2 = (wk_sb, bkc_sb, kT_sb, 2)
            QT3 = (wq_sb, bqc_sb, qT_sb, 3)
            KT3 = (wk_sb, bkc_sb, kT_sb, 3)
            # {unit: {kc: job}}; constraints: qT/kT(t2), kT(t3) before u4,
            # qT(t3) half h before unit 6+h; v chunk 16+j before pv(u4, j).
            a2_qk_slots = {
                0: {5: (*QT2, 0), 9: (*QT2, 1)},
                1: {5: (*KT2, 0), 9: (*KT2, 1)},
                2: {5: (*KT3, 0), 9: (*KT3, 1)},
                5: {5: (*QT3, 0)},
                6: {5: (*QT3, 1)},
            }
            a2_v_slots = {
                0: {12: 16, 14: 17},
                1: {12: 18, 14: 19},
                3: {11: 20, 12: 21, 13: 22, 14: 23},
                4: {0: 24, 1: 25, 2: 26, 3: 27, 5: 28, 6: 29, 7: 30, 8: 31},
            }
            # proj of unit u runs two units later (normalize(u) completes
            # early in u+1); u4/u5 split proj(u2); u7 carries u5 and u6.
            proj_slots = {
                2: {6: (0, 0), 8: (0, 1), 10: (0, 2), 12: (0, 3)},
                3: {6: (1, 0), 8: (1, 1), 10: (1, 2), 12: (1, 3)},
                4: {9: (2, 0), 11: (2, 1)},
                5: {7: (2, 2), 9: (2, 3), 11: (3, 0), 13: (3, 1)},
                6: {7: (3, 2), 9: (3, 3), 11: (4, 0), 13: (4, 1)},
                7: {2: (4, 2), 3: (4, 3), 6: (5, 0), 8: (5, 1), 10: (5, 2),
                    12: (5, 3), 7: (6, 0), 9: (6, 1), 11: (6, 2), 13: (6, 3)},
            }

            exp_tiles = {}          # (u, kc) -> exp SBUF tile
            pv_tiles = {}           # u -> [pv_h0, pv_h1] PSUM tiles

            def emit_scores(u, kc):
                b, qc = units[u]
                q_sl = slice(b * T + qc * 512, b * T + (qc + 1) * 512)
                sc = scores_ps.tile([128, 1024], f32, name="sc_ps", tag="ps")
                k_sl = slice(b * T + kc * 128, b * T + (kc + 1) * 128)
                for h in range(2):
                    nc.tensor.matmul(
                        sc[:, h * 512:(h + 1) * 512],
                        lhsT=kT_sb[h * 64:(h + 1) * 64, k_sl],
                        rhs=qT_sb[h * 64:(h + 1) * 64, q_sl],
                        start=True, stop=True)
                ex = exp_pool.tile([128, 1024], bf16, name="exp_sb", tag="exp_sb")
                # zero bias: exp(s+0); the biased ACT path runs ~1.5x faster
                nc.scalar.activation(ex[:], sc[:], EXP, bias=zeros_col[:])
                exp_tiles[(u, kc)] = ex

            def emit_pv(u, kc):
                b, qc = units[u]
                gkc = b * NKC + kc
                ex = exp_tiles.pop((u, kc))
                pv = pv_tiles[u]
                for h in range(2):
                    nc.tensor.matmul(
                        pv[h][:],
                        lhsT=v_sb[h][:, gkc * 128:(gkc + 1) * 128],
                        rhs=ex[:, h * 512:(h + 1) * 512],
                        start=(kc == 0), stop=(kc == NKC - 1),
                        skip_group_check=True)

            def emit_normalize(u):
                # D replicated on pv partitions 0:64, PV values on 64:128;
                # h1 written straight to attnT partitions 64:128.
                b, qc = units[u]
                pv = pv_tiles.pop(u)
                bcsb = bc_pool.tile([64, 1024], f32, name="bc_sb", tag="bc_sb")
                for h in range(2):
                    nc.vector.reciprocal_approx_fast(
                        out=bcsb[:, h * 512:(h + 1) * 512],
                        in_=pv[h][0:64, :])
                qcs = slice(qc * 512, (qc + 1) * 512)
                nc.vector.tensor_mul(
                    attnT[b][0:64, qcs], pv[0][64:128, :], bcsb[:, 0:512])
                nc.vector.tensor_mul(
                    attnT[b][64:128, qcs], pv[1][64:128, :], bcsb[:, 512:1024])

            for u in range(len(units)):
                pv_tiles[u] = [
                    pv_ps.tile([128, 512], f32, name=f"pv{h}", tag="pv")
                    for h in range(2)]
                for kc in range(NKC):
                    # drain previous unit's pv tail first, then own (lag 4)
                    if u > 0 and kc < 2:
                        emit_pv(u - 1, 12 + 2 * kc)
                        emit_pv(u - 1, 13 + 2 * kc)
                        if kc == 1:
                            emit_normalize(u - 1)
                    elif kc >= 4:
                        emit_pv(u, kc - 4)
                    emit_scores(u, kc)
                    job = a2_qk_slots.get(u, {}).get(kc)
                    if job is not None:
                        emit_a2_qk(*job)
                    tt = a2_v_slots.get(u, {}).get(kc)
                    if tt is not None:
                        emit_a2_v(tt)
                    pj = proj_slots.get(u, {}).get(kc)
                    if pj is not None:
                        pu, tci = pj
                        pb, pqc = units[pu]
                        emit_proj_tci(pb, pqc * 4 + tci)

            # drain: last unit's pv tail, then fine-grained per-128-token
            # pipeline: normalize slice (vector) -> proj pair into one free
            # [128,1024] scores-pool psum (scores pool is idle now) -> single
            # fused IDENTITY cast on the idle ACT engine -> DMA, alternating
            # HWDGE rings.  Keeps tensor/vector/ACT all busy in the tail.
            last = len(units) - 1
            b_l, qc_l = units[last]
            for kc in (12, 13, 14, 15):
                emit_pv(last, kc)
            pvl = pv_tiles.pop(last)
            bcl = bc_pool.tile([64, 1024], f32, name="bc_sb", tag="bc_sb")
            for i in range(4):
                cs = slice(i * 128, (i + 1) * 128)
                for h in range(2):
                    nc.vector.reciprocal_approx_fast(
                        out=bcl[:, h * 512 + i * 128:h * 512 + (i + 1) * 128],
                        in_=pvl[h][0:64, cs])
                acs = slice(qc_l * 512 + i * 128, qc_l * 512 + (i + 1) * 128)
                nc.vector.tensor_mul(
                    attnT[b_l][0:64, acs], pvl[0][64:128, cs],
                    bcl[:, i * 128:(i + 1) * 128])
                nc.vector.tensor_mul(
                    attnT[b_l][64:128, acs], pvl[1][64:128, cs],
                    bcl[:, 512 + i * 128:512 + (i + 1) * 128])
                tci = qc_l * 4 + i
                osb = out_pool.tile([128, 1024], bf16, name="out_sb", tag="out_sb")
                pps = scores_ps.tile([128, 1024], f32, name="dr_ps", tag="ps")
                for ncol in range(2):
                    nc.tensor.matmul(
                        pps[:, ncol * 512:(ncol + 1) * 512],
                        lhsT=attnT[b_l][:, tci * 128:(tci + 1) * 128],
                        rhs=wp_sb[:, ncol * 512:(ncol + 1) * 512],
                        start=True, stop=True)
                nc.vector.tensor_copy(osb[:, 0:512], pps[:, 0:512])
                nc.scalar.activation(osb[:, 512:1024], pps[:, 512:1024], IDENT)
                oq = nc.sync if i % 2 == 0 else nc.scalar
                oq.dma_start(
                    out_d[b_l * T + tci * 128: b_l * T + (tci + 1) * 128, :],
                    osb[:])

    nc.compile()
    return nc


def _prep_inputs(x, W_qkv, b_qkv, W_proj, b_proj):
    bf = ml_dtypes.bfloat16
    xT = np.ascontiguousarray(
        x.reshape(NTOK, C).T).astype(bf)
    in_maps = []
    for c in range(NCORES):
        cs = slice(c * CH, (c + 1) * CH)
        wq = np.ascontiguousarray(
            (W_qkv[:, c * CH:(c + 1) * CH] * SCALE)
            .reshape(8, 128, CH).transpose(1, 0, 2).reshape(128, C)).astype(bf)
        wk = np.ascontiguousarray(
            W_qkv[:, C + c * CH:C + (c + 1) * CH]
            .reshape(8, 128, CH).transpose(1, 0, 2).reshape(128, C)).astype(bf)
        wv = np.ascontiguousarray(
            W_qkv[:, 2 * C + c * CH:2 * C + (c + 1) * CH]
            .reshape(8, 128, CH).transpose(1, 0, 2).reshape(128, C)).astype(bf)
        wp = np.ascontiguousarray(W_proj[cs, :]).astype(bf)
        bqc = (b_qkv[c * CH:(c + 1) * CH] * SCALE).reshape(CH, 1).astype(np.float32)
        bkc = b_qkv[C + c * CH:C + (c + 1) * CH].reshape(CH, 1).astype(np.float32)
        bv = b_qkv[2 * C + c * CH:2 * C + (c + 1) * CH].reshape(1, CH).astype(bf)
        in_maps.append({
            "xT": xT, "wq": wq, "wk": wk, "wv": wv, "wp": wp,
            "bqc": bqc, "bkc": bkc, "bv": bv,
        })
    return in_maps


def _run(inputs, trace=False):
    from concourse import bass_utils
    if "nc" not in _CACHE:
        _CACHE["nc"] = _build()
    nc = _CACHE["nc"]
    in_maps = _prep_inputs(
        np.asarray(inputs["x"], np.float32),
        np.asarray(inputs["W_qkv"], np.float32),
        np.asarray(inputs["b_qkv"], np.float32),
        np.asarray(inputs["W_proj"], np.float32),
        np.asarray(inputs["b_proj"], np.float32),
    )
    br = bass_utils.run_bass_kernel_spmd(
        nc, in_maps, core_ids=list(range(NCORES)), trace=trace)
    partial = np.zeros((NTOK, C), np.float64)
    for r in br.results:
        partial += np.asarray(r["out"]).astype(np.float64)
    out = (partial + np.asarray(inputs["b_proj"], np.float64)[None, :]).astype(
        np.float32).reshape(B, T, C)
    return out, br


def kernel(**inputs) -> np.ndarray:
    out, _ = _run(inputs, trace=False)
    return out


# revision 17
# speedup vs baseline: 1.0051x; 1.0051x over previous
"""Trainium2 Bass kernel: 16-head MHA (B=2, T=2048, D=1024), head-TP over 8 cores.

Per core c: heads 2c, 2c+1 (128 channels). Device computes x@Wqkv(+b) for its
head slice, scoresT=K@Q^T (scale folded into Wq), exp via ACT, P@V with an
appended ones-column producing the softmax denominator for free, normalize,
then partial proj = attn_c @ Wproj[c-slice]. Host sums the 8 partials + b_proj.

v2: proj deferred one pipeline unit (off the normalize critical path), h1
normalize written directly to attnT partitions 64:128 (no SBUF-SBUF DMA),
a2 bias-adds moved off the exp-saturated ACT engine to gpsimd, proj PSUM
drains split vector/gpsimd, startup DMAs reordered onto the two HWDGE rings,
out-tile DMAs alternate the HWDGE rings.
"""

import numpy as np
import ml_dtypes
from contextlib import ExitStack

B, T, C = 2, 2048, 1024
H, DH = 16, 64
NCORES = 8
CH = 128               # channels per core = 2 heads
NTOK = B * T           # 4096
NKC = T // 128         # 16 key chunks per batch
NQC = T // 512         # 4 query chunks per batch
SCALE = DH ** -0.5

_CACHE = {}


def _build(debug=False):
    import concourse.bass as bass  # noqa: F401
    import concourse.bacc as bacc
    import concourse.mybir as mybir
    import concourse.tile as tile

    f32 = mybir.dt.float32
    bf16 = mybir.dt.bfloat16
    EXP = mybir.ActivationFunctionType.Exp
    IDENT = mybir.ActivationFunctionType.Identity

    # Bacc (not Bass): its compile() runs move_matmul_waits_to_ldweights +
    # generate_event_semaphores, without which walrus rejects matmuls
    # carrying 2 sync waits ("Too many sync wait commands").
    nc = bacc.Bacc("TRN2", target_bir_lowering=False, debug=False)
    xT_d = nc.declare_dram_parameter("xT", [C, NTOK], bf16, isOutput=False)
    wq_d = nc.declare_dram_parameter("wq", [128, C], bf16, isOutput=False)
    wk_d = nc.declare_dram_parameter("wk", [128, C], bf16, isOutput=False)
    wv_d = nc.declare_dram_parameter("wv", [128, C], bf16, isOutput=False)
    wp_d = nc.declare_dram_parameter("wp", [CH, C], bf16, isOutput=False)
    bqc_d = nc.declare_dram_parameter("bqc", [CH, 1], f32, isOutput=False)
    bkc_d = nc.declare_dram_parameter("bkc", [CH, 1], f32, isOutput=False)
    bv_d = nc.declare_dram_parameter("bv", [1, CH], bf16, isOutput=False)
    out_d = nc.declare_dram_parameter("out", [NTOK, C], bf16, isOutput=True)

    with tile.TileContext(nc) as tc, ExitStack() as ctx:
        ep = ctx.enter_context

        # ---------------- persistent SBUF ----------------
        xT_pool = ep(tc.tile_pool(name="xT", bufs=8))
        xT_sb = [xT_pool.tile([128, NTOK], bf16, name=f"xT{k}", tag="xT") for k in range(8)]
        w_pool = ep(tc.tile_pool(name="w", bufs=4))
        wq_sb = w_pool.tile([128, C], bf16, tag="wq")
        wk_sb = w_pool.tile([128, C], bf16, tag="wk")
        wv_sb = w_pool.tile([128, C], bf16, tag="wv")
        wp_sb = w_pool.tile([CH, C], bf16, tag="wp")
        b_pool = ep(tc.tile_pool(name="bias", bufs=1))
        bqc_sb = b_pool.tile([CH, 1], f32, tag="bqc")
        bkc_sb = b_pool.tile([CH, 1], f32, tag="bkc")
        bv_sb = b_pool.tile([1, CH], bf16, tag="bv")
        bv_bc = b_pool.tile([128, CH], bf16, tag="bv_bc")
        const_pool = ep(tc.tile_pool(name="const", bufs=2))
        ones_bf = const_pool.tile([1, 512], bf16, tag="ones_bf")
        qk_pool = ep(tc.tile_pool(name="qk", bufs=2))
        qT_sb = qk_pool.tile([CH, NTOK], bf16, tag="qT")
        kT_sb = qk_pool.tile([CH, NTOK], bf16, tag="kT")
        v_pool = ep(tc.tile_pool(name="v", bufs=2))
        # per head: B*NKC chunks of [128 keys, 64 ones cols | 64 feats]; the
        # ones cols make the PV matmul replicate the softmax denominator onto
        # output partitions 0:64 for free (recip reads physical partition 0).
        v_sb = [v_pool.tile([128, B * NKC * 128], bf16, name=f"v{h}", tag="v") for h in range(2)]
        attn_pool = ep(tc.tile_pool(name="attn", bufs=2))
        attnT = [attn_pool.tile([CH, T], bf16, name=f"attnT{b}", tag="attnT") for b in range(B)]
        exp_pool = ep(tc.tile_pool(name="exp", bufs=6))
        bc_pool = ep(tc.tile_pool(name="bcsb", bufs=2))
        out_pool = ep(tc.tile_pool(name="outsb", bufs=4))

        # ---------------- memsets first (no DMA deps) ----------------
        nc.vector.memset(ones_bf[:], 1.0)
        # whole v tile to 1.0; value cols 0:64 of each chunk overwritten later
        nc.vector.memset(v_sb[0][:], 1.0)
        nc.gpsimd.memset(v_sb[1][:], 1.0)

        # ---------------- load inputs ----------------
        # x loads as [128, 2048] per-batch blocks: 4KB contiguous lines per
        # partition (near-peak DMA efficiency) and only 16 transfers.  Batch 0
        # first (phase A consumes it), batch 1 right after (a2 jobs need it
        # from ~45us).  All on the two HWDGE rings; weights interleaved.
        def xblock(q, k, b):
            q.dma_start(
                xT_sb[k][:, b * T:(b + 1) * T],
                xT_d[k * 128:(k + 1) * 128, b * T:(b + 1) * T])

        nc.scalar.dma_start(bv_sb[:], bv_d[:])
        nc.sync.dma_start(wq_sb[:, 0:512], wq_d[:, 0:512])
        nc.scalar.dma_start(wq_sb[:, 512:1024], wq_d[:, 512:1024])
        nc.scalar.dma_start(bqc_sb[:], bqc_d[:])
        nc.scalar.dma_start(bkc_sb[:], bkc_d[:])
        for k in range(8):
            xblock(nc.sync if k % 2 == 0 else nc.scalar, k, 0)
        nc.sync.dma_start(wk_sb[:], wk_d[:])
        nc.scalar.dma_start(wv_sb[:], wv_d[:])
        for k in range(8):
            xblock(nc.sync if k % 2 == 0 else nc.scalar, k, 1)
        nc.sync.dma_start(wp_sb[:], wp_d[:])

        # ---------------- phase A: qkv projections ----------------
        with tc.tile_pool(name="qk_ps", bufs=2, space="PSUM") as qk_psp, \
             tc.tile_pool(name="v_psp", bufs=4, space="PSUM") as v_psp:
            # bv broadcast [128, CH] built once (v bias folded into DVE copy)
            bvps = v_psp.tile([128, CH], f32, name="bv_ps", tag="v_ps")
            nc.tensor.matmul(bvps[:], lhsT=ones_bf[:, :128], rhs=bv_sb[:],
                             start=True, stop=True)
            nc.vector.tensor_copy(bv_bc[:], bvps[:])
            # per 1024-token group: q chunk, k chunk, then 8 v chunks.
            # only batch 0 (t=0,1) here; batch 1's qkv is interleaved into the
            # ACT-bound b=0 attention window below.
            for t in range(2):
                for w_sb, bias_col, dst in ((wq_sb, bqc_sb, qT_sb),
                                            (wk_sb, bkc_sb, kT_sb)):
                    ps = qk_psp.tile([CH, 1024], f32, name="qk_ps", tag="qk_ps")
                    for half in range(2):
                        for k in range(8):
                            nc.tensor.matmul(
                                ps[:, half * 512:(half + 1) * 512],
                                lhsT=w_sb[:, k * 128:(k + 1) * 128],
                                rhs=xT_sb[k][:, t * 1024 + half * 512:
                                              t * 1024 + (half + 1) * 512],
                                start=(k == 0), stop=(k == 7))
                    # bias add fused into PSUM->SBUF copy on the (here idle)
                    # ACT engine
                    nc.scalar.activation(
                        dst[:, t * 1024:(t + 1) * 1024], ps[:], IDENT,
                        bias=bias_col[:])
                for tt in range(t * 8, (t + 1) * 8):
                    ps = v_psp.tile([128, CH], f32, name="v_ps", tag="v_ps")
                    for k in range(8):
                        nc.tensor.matmul(
                            ps[:], lhsT=xT_sb[k][:, tt * 128:(tt + 1) * 128],
                            rhs=wv_sb[:, k * 128:(k + 1) * 128],
                            start=(k == 0), stop=(k == 7))
                    for h in range(2):
                        nc.vector.tensor_add(
                            v_sb[h][:, tt * 128 + 64:(tt + 1) * 128],
                            ps[:, h * 64:(h + 1) * 64],
                            bv_bc[:, h * 64:(h + 1) * 64])

        # ---------------- phase B: attention (+ interleaved proj of prev unit)
        with tc.tile_pool(name="scores_ps", bufs=2, space="PSUM") as scores_ps, \
             tc.tile_pool(name="pv_ps", bufs=2, space="PSUM") as pv_ps, \
             tc.tile_pool(name="proj_ps", bufs=2, space="PSUM") as proj_ps:

            def emit_proj_tci(b, tci):
                """proj partial for one 128-token chunk: out += attn @ Wp_c"""
                osb = out_pool.tile([128, 1024], bf16, name="out_sb", tag="out_sb")
                for ncol in range(2):
                    pps = proj_ps.tile([128, 512], f32, name="proj_ps", tag="pj")
                    nc.tensor.matmul(
                        pps[:],
                        lhsT=attnT[b][:, tci * 128:(tci + 1) * 128],
                        rhs=wp_sb[:, ncol * 512:(ncol + 1) * 512],
                        start=True, stop=True)
                    nc.vector.tensor_copy(
                        osb[:, ncol * 512:(ncol + 1) * 512], pps[:])
                oq = nc.gpsimd if tci % 2 == 0 else nc.sync
                oq.dma_start(
                    out_d[b * T + tci * 128: b * T + (tci + 1) * 128, :], osb[:])

            def emit_a2_qk(w_sb, bias_col, dst, t, half):
                """one [CH,512] half of a batch-1 q/k projection group"""
                ps = proj_ps.tile([128, 512], f32, name="a2_ps", tag="pj")
                sl = slice(t * 1024 + half * 512, t * 1024 + (half + 1) * 512)
                for k in range(8):
                    nc.tensor.matmul(
                        ps[:], lhsT=w_sb[:, k * 128:(k + 1) * 128],
                        rhs=xT_sb[k][:, sl], start=(k == 0), stop=(k == 7))
                # bias add on vector: ACT is exp-saturated in this window
                # (gpsimd cannot read PSUM)
                nc.vector.tensor_scalar_add(dst[:, sl], ps[:], bias_col[:])

            def emit_a2_v(tt):
                """one [128,CH] batch-1 v chunk"""
                ps = proj_ps.tile([128, 512], f32, name="a2v_ps", tag="pj")
                for k in range(8):
                    nc.tensor.matmul(
                        ps[:, 0:CH], lhsT=xT_sb[k][:, tt * 128:(tt + 1) * 128],
                        rhs=wv_sb[:, k * 128:(k + 1) * 128],
                        start=(k == 0), stop=(k == 7))
                for h in range(2):
                    nc.vector.tensor_add(
                        v_sb[h][:, tt * 128 + 64:(tt + 1) * 128],
                        ps[:, h * 64:(h + 1) * 64],
                        bv_bc[:, h * 64:(h + 1) * 64])

            # Units u0..u7 = (b, qc).  Globally software-pipelined stream:
            # PV of unit u lags its scores by 4 kc and its last 4 PVs drain
            # in the first two iterations of unit u+1; exp tiles buffer 6
            # deep so the ACT engine never starves across unit boundaries.
            # a2 (batch-1 qkv) and deferred proj jobs are distributed so
            # every unit's tensor work exceeds its 16us of exp on ACT.
            units = [(b, qc) for b in range(B) for qc in range(NQC)]
            QT2 = (wq_sb, bqc_sb, qT_sb, 2)
            KT2 = (wk_sb, bkc_sb, kT_sb, 2)
            QT3 = (wq_sb, bqc_sb, qT_sb, 3)
            KT3 = (wk_sb, bkc_sb, kT_sb, 3)
            # {unit: {kc: job}}; constraints: qT/kT(t2), kT(t3) before u4,
            # qT(t3) half h before unit 6+h; v chunk 16+j before pv(u4, j).
            a2_qk_slots = {
                0: {5: (*QT2, 0), 9: (*QT2, 1)},
                1: {5: (*KT2, 0), 9: (*KT2, 1)},
                2: {5: (*KT3, 0), 9: (*KT3, 1)},
                5: {5: (*QT3, 0)},
                6: {5: (*QT3, 1)},
            }
            a2_v_slots = {
                0: {12: 16, 14: 17},
                1: {12: 18, 14: 19},
                3: {11: 20, 12: 21, 13: 22, 14: 23},
                4: {0: 24, 1: 25, 2: 26, 3: 27, 5: 28, 6: 29, 7: 30, 8: 31},
            }
            # proj of unit u runs two units later (normalize(u) completes
            # early in u+1); u4/u5 split proj(u2); u7 carries u5 and u6.
            proj_slots = {
                2: {6: (0, 0), 8: (0, 1), 10: (0, 2), 12: (0, 3)},
                3: {6: (1, 0), 8: (1, 1), 10: (1, 2), 12: (1, 3)},
                4: {9: (2, 0), 11: (2, 1)},
                5: {7: (2, 2), 9: (2, 3), 11: (3, 0), 13: (3, 1)},
                6: {7: (3, 2), 9: (3, 3), 11: (4, 0), 13: (4, 1)},
                7: {2: (4, 2), 3: (4, 3), 6: (5, 0), 8: (5, 1), 10: (5, 2),
                    12: (5, 3), 7: (6, 0), 9: (6, 1), 11: (6, 2), 13: (6, 3)},
            }

            exp_tiles = {}          # (u, kc) -> exp SBUF tile
            pv_tiles = {}           # u -> [pv_h0, pv_h1] PSUM tiles

            def emit_scores(u, kc):
                b, qc = units[u]
                q_sl = slice(b * T + qc * 512, b * T + (qc + 1) * 512)
                sc = scores_ps.tile([128, 1024], f32, name="sc_ps", tag="ps")
                k_sl = slice(b * T + kc * 128, b * T + (kc + 1) * 128)
                for h in range(2):
                    nc.tensor.matmul(
                        sc[:, h * 512:(h + 1) * 512],
                        lhsT=kT_sb[h * 64:(h + 1) * 64, k_sl],
                        rhs=qT_sb[h * 64:(h + 1) * 64, q_sl],
                        start=True, stop=True)
                ex = exp_pool.tile([128, 1024], bf16, name="exp_sb", tag="exp_sb")
                nc.scalar.activation(ex[:], sc[:], EXP)
                exp_tiles[(u, kc)] = ex

            def emit_pv(u, kc):
                b, qc = units[u]
                gkc = b * NKC + kc
                ex = exp_tiles.pop((u, kc))
                pv = pv_tiles[u]
                for h in range(2):
                    nc.tensor.matmul(
                        pv[h][:],
                        lhsT=v_sb[h][:, gkc * 128:(gkc + 1) * 128],
                        rhs=ex[:, h * 512:(h + 1) * 512],
                        start=(kc == 0), stop=(kc == NKC - 1),
                        skip_group_check=True)

            def emit_normalize(u):
                # D replicated on pv partitions 0:64, PV values on 64:128;
                # h1 written straight to attnT partitions 64:128.
                b, qc = units[u]
                pv = pv_tiles.pop(u)
                bcsb = bc_pool.tile([64, 1024], f32, name="bc_sb", tag="bc_sb")
                for h in range(2):
                    nc.vector.reciprocal_approx_fast(
                        out=bcsb[:, h * 512:(h + 1) * 512],
                        in_=pv[h][0:64, :])
                qcs = slice(qc * 512, (qc + 1) * 512)
                nc.vector.tensor_mul(
                    attnT[b][0:64, qcs], pv[0][64:128, :], bcsb[:, 0:512])
                nc.vector.tensor_mul(
                    attnT[b][64:128, qcs], pv[1][64:128, :], bcsb[:, 512:1024])

            for u in range(len(units)):
                pv_tiles[u] = [
                    pv_ps.tile([128, 512], f32, name=f"pv{h}", tag="pv")
                    for h in range(2)]
                for kc in range(NKC):
                    # drain previous unit's pv tail first, then own (lag 4)
                    if u > 0 and kc < 2:
                        emit_pv(u - 1, 12 + 2 * kc)
                        emit_pv(u - 1, 13 + 2 * kc)
                        if kc == 1:
                            emit_normalize(u - 1)
                    elif kc >= 4:
                        emit_pv(u, kc - 4)
                    emit_scores(u, kc)
                    job = a2_qk_slots.get(u, {}).get(kc)
                    if job is not None:
                        emit_a2_qk(*job)
                    tt = a2_v_slots.get(u, {}).get(kc)
                    if tt is not None:
                        emit_a2_v(tt)
                    pj = proj_slots.get(u, {}).get(kc)
                    if pj is not None:
                        pu, tci = pj
                        pb, pqc = units[pu]
                        emit_proj_tci(pb, pqc * 4 + tci)

            # drain: last unit's pv tail, then fine-grained per-128-token
            # pipeline: normalize slice (vector) -> proj pair into one free
            # [128,1024] scores-pool psum (scores pool is idle now) -> single
            # fused IDENTITY cast on the idle ACT engine -> DMA, alternating
            # HWDGE rings.  Keeps tensor/vector/ACT all busy in the tail.
            last = len(units) - 1
            b_l, qc_l = units[last]
            for kc in (12, 13, 14, 15):
                emit_pv(last, kc)
            pvl = pv_tiles.pop(last)
            bcl = bc_pool.tile([64, 1024], f32, name="bc_sb", tag="bc_sb")
            for i in range(4):
                cs = slice(i * 128, (i + 1) * 128)
                for h in range(2):
                    nc.vector.reciprocal_approx_fast(
                        out=bcl[:, h * 512 + i * 128:h * 512 + (i + 1) * 128],
                        in_=pvl[h][0:64, cs])
                acs = slice(qc_l * 512 + i * 128, qc_l * 512 + (i + 1) * 128)
                nc.vector.tensor_mul(
                    attnT[b_l][0:64, acs], pvl[0][64:128, cs],
                    bcl[:, i * 128:(i + 1) * 128])
                nc.vector.tensor_mul(
                    attnT[b_l][64:128, acs], pvl[1][64:128, cs],
                    bcl[:, 512 + i * 128:512 + (i + 1) * 128])
                tci = qc_l * 4 + i
                osb = out_pool.tile([128, 1024], bf16, name="out_sb", tag="out_sb")
                pps = scores_ps.tile([128, 1024], f32, name="dr_ps", tag="ps")
                for ncol in range(2):
                    nc.tensor.matmul(
                        pps[:, ncol * 512:(ncol + 1) * 512],
                        lhsT=attnT[b_l][:, tci * 128:(tci + 1) * 128],
                        rhs=wp_sb[:, ncol * 512:(ncol + 1) * 512],
                        start=True, stop=True)
                nc.vector.tensor_copy(osb[:, 0:512], pps[:, 0:512])
                nc.scalar.activation(osb[:, 512:1024], pps[:, 512:1024], IDENT)
                oq = nc.sync if i % 2 == 0 else nc.scalar
                oq.dma_start(
                    out_d[b_l * T + tci * 128: b_l * T + (tci + 1) * 128, :],
                    osb[:])

    nc.compile()
    return nc


def _prep_inputs(x, W_qkv, b_qkv, W_proj, b_proj):
    bf = ml_dtypes.bfloat16
    xT = np.ascontiguousarray(
        x.reshape(NTOK, C).T).astype(bf)
    in_maps = []
    for c in range(NCORES):
        cs = slice(c * CH, (c + 1) * CH)
        wq = np.ascontiguousarray(
            (W_qkv[:, c * CH:(c + 1) * CH] * SCALE)
            .reshape(8, 128, CH).transpose(1, 0, 2).reshape(128, C)).astype(bf)
        wk = np.ascontiguousarray(
            W_qkv[:, C + c * CH:C + (c + 1) * CH]
            .reshape(8, 128, CH).transpose(1, 0, 2).reshape(128, C)).astype(bf)
        wv = np.ascontiguousarray(
            W_qkv[:, 2 * C + c * CH:2 * C + (c + 1) * CH]
            .reshape(8, 128, CH).transpose(1, 0, 2).reshape(128, C)).astype(bf)
        wp = np.ascontiguousarray(W_proj[cs, :]).astype(bf)
        bqc = (b_qkv[c * CH:(c + 1) * CH] * SCALE).reshape(CH, 1).astype(np.float32)
        bkc = b_qkv[C + c * CH:C + (c + 1) * CH].reshape(CH, 1).astype(np.float32)
        bv = b_qkv[2 * C + c * CH:2 * C + (c + 1) * CH].reshape(1, CH).astype(bf)
        in_maps.append({
            "xT": xT, "wq": wq, "wk": wk, "wv": wv, "wp": wp,
            "bqc": bqc, "bkc": bkc, "bv": bv,
        })
    return in_maps


def _run(inputs, trace=False):
    from concourse import bass_utils
    if "nc" not in _CACHE:
        _CACHE["nc"] = _build()
    nc = _CACHE["nc"]
    in_maps = _prep_inputs(
        np.asarray(inputs["x"], np.float32),
        np.asarray(inputs["W_qkv"], np.float32),
        np.asarray(inputs["b_qkv"], np.float32),
        np.asarray(inputs["W_proj"], np.float32),
        np.asarray(inputs["b_proj"], np.float32),
    )
    br = bass_utils.run_bass_kernel_spmd(
        nc, in_maps, core_ids=list(range(NCORES)), trace=trace)
    partial = np.zeros((NTOK, C), np.float64)
    for r in br.results:
        partial += np.asarray(r["out"]).astype(np.float64)
    out = (partial + np.asarray(inputs["b_proj"], np.float64)[None, :]).astype(
        np.float32).reshape(B, T, C)
    return out, br


def kernel(**inputs) -> np.ndarray:
    out, _ = _run(inputs, trace=False)
    return out
